# revision 36
# baseline (speedup 1.0000x reference)
"""ExemplarAttention Trainium2 kernel (8 NeuronCores, exemplar-sharded).

logits[b,c] = gamma * log(sum_{n:label[n]=c} exp(-beta * sum_k w_k (x[b,k]-e[n,k])^2) + eps)

Strategy (v5 — exemplar/N-sharded, transposed GEMM, 16-tile packing):
  - Shard the N=16384 exemplars across the 8 cores (2048 each, a
    contiguous block of the (label, e2w)-sorted order) and replicate the
    batch. Per-core DMA is ~1.6MB (vs 8MB for batch sharding).
  - Transposed GEMM: psum[n_part, b_free] = S * cross, exemplar features
    stationary, S*x*w moving, fp8 DoubleRow (K=2x256). The per-exemplar
    -beta*e2w rides the ScalarE activation bias (per-partition), so
    there is no aug matmul pass at all.
  - Each partition holds 4 class-pure "cells" of sizes {5,6,4,1}
    (tiles 0-4 / 5-10 / 11-14 / 15), one per accumulator group: 16
    tiles cover 2048 exemplars with zero padding. Cells take
    consecutive exemplars in e2w order, so psum-chunk pairs share one
    bias value (pair-mean, ~1e-4 relative error).
  - ScalarE exps whole psum chunks ([128,2048] pairs / [128,1024]
    singles); VectorE folds them into per-cell bf16 accumulators (2x
    mode); the first/last tiles are written by ScalarE directly. The
    four accumulators stream out as they finalize; the host does the
    per-class gather, exp(-beta*x2w[b]) and gamma*log(.+eps) in f64.
  - ScalarE's exp (16 tiles x 1024 cols @ 1 col/cycle @1.2GHz = 13.7us
    + per-instr overhead) is the irreducible bottleneck; PE (~7us of
    fp8 matmul at the observed 1.2GHz effective rate), VectorE (~9us)
    and DMA (~5us) hide under it. Input DMA is issued in strict
    priority order on one ring: the queues drain ~concurrently at
    ~300GB/s, so whatever chunk0 needs must be enqueued first.
"""

import os
from contextlib import ExitStack

import numpy as np

B, N, D, C = 1024, 16384, 512, 10
NCORES = 8
NLOC = N // NCORES           # 2048 exemplars per core
NT = 16                      # exemplar slots (tiles) per partition
NG = 2                       # DoubleRow K-groups (K=256 each)
NTILE = 512                  # matmul free dim (1 psum bank)
EPS = 1e-9
S_SCALE = 128.0              # fp8 scale applied to x*w
PAD_BIAS = -100.0            # bias for all-padding chunks: exp(-100) == 0

# class-pure cells per partition: (first tile, size); one per acc group.
CELLS = ((0, 5), (5, 6), (11, 4), (15, 1))
# psum chunks (never straddle a cell boundary): singles up front (their
# 4-matmul dependency keeps Act fed even while the PE clock is still in
# its slow post-idle state), then pairs, a single at the end.
CHUNKS = ((0,), (1,), (2,), (3, 4), (5, 6), (7, 8), (9, 10), (11, 12),
          (13, 14), (15,))
# output buffers: cell g2 is split across two buffers (3 and 4) so the
# tail only waits on tiles 13-14's accumulator, not all of g2.
NACC = 5
ACC_OF_TILE_MAP = {**{t: 0 for t in range(5)}, **{t: 1 for t in range(5, 11)},
                   11: 2, 12: 2, 13: 4, 14: 4}
ACC_CELL = (0, 1, 2, 3, 2)   # output buffer -> cell (class/pad bookkeeping)
ACT_DIRECT = {0: 0, 15: 3}   # tile -> acc written by ScalarE directly
ACC_LAST_CHUNK = {0: 3, 1: 6, 2: 7, 4: 8}  # acc -> final after this chunk
GROUP_OF_CHUNK = tuple(
    next(gi for gi, (t0, sz) in enumerate(CELLS) if t0 <= ts[0] < t0 + sz)
    for ts in CHUNKS)


def _acc_of_tile(t):
    return ACC_OF_TILE_MAP[t]


N_WARMUP_MM = 18

_prog_cache = {}


def _np_dt(mybir, name):
    return mybir.dt.np(getattr(mybir.dt, name))


def _build_program(act_scale):
    import concourse.bass as bass  # noqa: F401
    import concourse.tile as tile
    from concourse import bacc, mybir

    fp8 = mybir.dt.float8e4
    bf16 = mybir.dt.bfloat16
    f32 = mybir.dt.float32
    DR = mybir.MatmulPerfMode.DoubleRow
    ADD = mybir.AluOpType.add

    nc = bacc.Bacc("TRN2", target_bir_lowering=False, debug=False,
                   num_devices=NCORES)

    e_d = nc.dram_tensor("e_t", [128, NT, NG, 2, 128], fp8,
                         kind="ExternalInput").ap()
    xw_d = nc.dram_tensor("xw_t", [128, NG, 2, B], fp8,
                          kind="ExternalInput").ap()
    bias_d = nc.dram_tensor("bias", [128, len(CHUNKS)], f32,
                            kind="ExternalInput").ap()
    out_d = nc.dram_tensor("acc", [NACC, 128, B], bf16,
                           kind="ExternalOutput").ap()

    with tile.TileContext(nc) as tc, ExitStack() as ctx:
        singles = ctx.enter_context(tc.tile_pool(name="singles", bufs=1))
        psum_pool = ctx.enter_context(tc.tile_pool(name="ps", bufs=2,
                                                   space="PSUM"))
        tmp_pool = ctx.enter_context(tc.tile_pool(name="tmp", bufs=3))

        xw_sb = singles.tile([128, NG, 2, B], fp8)
        e_sb = singles.tile([128, NT, NG, 2, 128], fp8)
        bias_sb = singles.tile([128, len(CHUNKS)], f32)
        accs = [singles.tile([128, B], bf16, name=f"acc{i}")
                for i in range(NACC)]

        # Warmup matmul operands: small memset tile, no DMA dependency.
        dmy = singles.tile([128, 2, 256], fp8)
        nc.vector.memset(dmy[:, :, :], 0.0)

        # Input DMA: the window is descriptor-count bound (~160ns per
        # partition-row descriptor per queue, 16 queues, FIFO), so use
        # few fat transfers, earliest-needed first, on two rings.
        nc.scalar.dma_start(out=bias_sb[:, :], in_=bias_d[:, :])
        nc.sync.dma_start(out=xw_sb[:, :, :, :], in_=xw_d[:, :, :, :])
        nc.gpsimd.dma_start(out=e_sb[:, 0:3, :, :, :],
                            in_=e_d[:, 0:3, :, :, :])
        # In-flight DMAs share the 16 queues round-robin, so a transfer
        # issued early steals bandwidth from the head-critical xw/e0-2
        # set. 1-descriptor dummy DMAs (~620ns of issue time each) delay
        # the bulk exemplar transfers until the critical set has drained.
        dscr = singles.tile([1, 4], f32)
        nc.sync.dma_start(out=dscr[0:1, 0:1], in_=bias_d[0:1, 0:1])
        nc.sync.dma_start(out=dscr[0:1, 1:2], in_=bias_d[0:1, 0:1])
        nc.sync.dma_start(out=dscr[0:1, 2:3], in_=bias_d[0:1, 0:1])
        nc.sync.dma_start(out=e_sb[:, 3:9, :, :, :],
                          in_=e_d[:, 3:9, :, :, :])
        nc.sync.dma_start(out=dscr[0:1, 3:4], in_=bias_d[0:1, 0:1])
        nc.sync.dma_start(out=e_sb[:, 9:NT, :, :, :],
                          in_=e_d[:, 9:NT, :, :, :])

        # Warmup: keep the PE busy while the head DMAs land.
        ps0 = psum_pool.tile([128, 2048], f32, tag="ps", name="ps0")
        for _ in range(N_WARMUP_MM):
            nc.tensor.matmul(ps0[:, 0:256], lhsT=dmy[:, :, 0:128],
                             rhs=dmy[:, :, :], start=True, stop=True,
                             perf_mode=DR)

        acc_touched = [False] * NACC
        for j, tiles_ in enumerate(CHUNKS):
            ps = ps0 if j == 0 else psum_pool.tile([128, 2048], f32, tag="ps",
                                                   name=f"ps{j}")
            for ti, t in enumerate(tiles_):
                for g in range(NG):
                    for h in range(2):
                        c0 = ti * 1024 + h * NTILE
                        nc.tensor.matmul(
                            ps[:, c0:c0 + NTILE],
                            lhsT=e_sb[:, t, g, :, :],
                            rhs=xw_sb[:, g, :, h * NTILE:(h + 1) * NTILE],
                            start=(g == 0), stop=(g == NG - 1),
                            perf_mode=DR)

            t0 = tiles_[0]
            w = 1024 * len(tiles_)
            # single-tile chunks that first-touch their acc: ScalarE
            # writes the exp straight into the accumulator, no VectorE op.
            direct = len(tiles_) == 1 and t0 in ACT_DIRECT
            tmp = (accs[ACT_DIRECT[t0]] if direct
                   else tmp_pool.tile([128, 2048], bf16, tag="tmp"))
            nc.scalar.activation(
                out=tmp[:, 0:w],
                in_=ps[:, 0:w],
                func=mybir.ActivationFunctionType.Exp,
                bias=bias_sb[:, j:j + 1],
                scale=act_scale,
            )
            if direct:
                acc_touched[ACT_DIRECT[t0]] = True
                if t0 == 15:  # final acc: stream out immediately
                    nc.sync.dma_start(out=out_d[ACT_DIRECT[t0], :, :],
                                      in_=tmp[:, :])
                continue
            for ti, t in enumerate(tiles_):
                a = accs[_acc_of_tile(t)]
                sl = tmp[:, ti * 1024:(ti + 1) * 1024]
                if not acc_touched[_acc_of_tile(t)]:
                    acc_touched[_acc_of_tile(t)] = True
                    nc.vector.tensor_scalar_mul(a[:, :], sl, 1.0)
                else:
                    nc.vector.tensor_tensor(out=a[:, :], in0=a[:, :], in1=sl,
                                            op=ADD)
            for ai, jlast in ACC_LAST_CHUNK.items():
                if j == jlast:
                    nc.sync.dma_start(out=out_d[ai, :, :], in_=accs[ai][:, :])

    nc.compile()
    return nc


def _decompose(m):
    """Per-class cell counts per group {5,6,4,1}: 5a+6b+4d+e = m_c with
    each column summing to 128. Returns (C,4)."""
    m = np.asarray(m, dtype=np.int64)
    assert m.sum() == NT * 128
    k = m // 16
    e = m - 15 * k
    deficit = 128 - int(k.sum())
    if deficit < 0:
        for c in np.argsort(-k):
            while deficit < 0 and k[c] > 0:
                k[c] -= 1
                e[c] += 15
                deficit += 1
    else:
        while deficit > 0:
            c = int(np.argmax(e))
            if e[c] < 16:
                break
            k[c] += 1
            e[c] -= 15
            deficit -= 1
    if (k.sum() == 128 and e.sum() == 128 and (e >= 0).all()
            and (k >= 0).all()):
        return np.stack([k, k, k, e], axis=1)
    # general fallback: assign each group's 128 cells by largest
    # remainder of the outstanding per-class counts, then repair any
    # class whose leftover went negative by stealing unit cells.
    rem = m.astype(np.int64).copy()
    cols = []
    for sz in (5, 6, 4):
        frac = rem / max(rem.sum(), 1) * 128
        cnt = np.floor(frac).astype(np.int64)
        for c in np.argsort(-(frac - cnt)):
            if cnt.sum() >= 128:
                break
            cnt[c] += 1
        while True:  # cap at what the class can still supply
            over = np.where(cnt * sz > rem)[0]
            if len(over) == 0:
                break
            for c in over:
                cnt[c] = rem[c] // sz
            for c in np.argsort(-(rem - cnt * sz)):
                if cnt.sum() >= 128:
                    break
                if (cnt[c] + 1) * sz <= rem[c]:
                    cnt[c] += 1
            if cnt.sum() < 128:
                raise AssertionError(f"cell packing infeasible: {m}")
        cols.append(cnt)
        rem = rem - cnt * sz
    assert (rem >= 0).all() and rem.sum() == 128
    cols.append(rem)
    return np.stack(cols, axis=1)


def _pack(labels, e2w, beta):
    """Pack exemplars into per-core [128, NT] grids of class-pure,
    e2w-sorted cells. Returns per-core (grid, bias[128,nchunks] f32,
    pad_const[NACC,128] f64, cls[128,NACC])."""
    import concourse.mybir as mybir
    bf16 = _np_dt(mybir, "bfloat16")

    order = np.lexsort((e2w, labels))
    blocks = order.reshape(NCORES, NLOC)
    out = []
    for cid in range(NCORES):
        ids = blocks[cid]
        m = np.bincount(labels[ids], minlength=C)
        cells = _decompose(m)
        assert cells is not None, f"cell decomposition failed: {m}"
        starts = np.concatenate([[0], np.cumsum(m)])
        pos = starts[:-1].copy()
        grid = np.full((128, NT), -1, dtype=np.int64)
        cls = np.full((128, len(CELLS)), -1, dtype=np.int64)
        for gi, (t0, sz) in enumerate(CELLS):
            p = 0
            for c in range(C):
                for _ in range(int(cells[c, gi])):
                    grid[p, t0:t0 + sz] = ids[pos[c]:pos[c] + sz]
                    cls[p, gi] = c
                    pos[c] += sz
                    p += 1
            assert p == 128
        assert (pos == starts[1:]).all()

        bias = np.full((128, len(CHUNKS)), PAD_BIAS, dtype=np.float64)
        npad = np.zeros((128, len(CHUNKS)), dtype=np.int64)
        e2w_g = np.where(grid >= 0, e2w[grid.clip(0)], np.nan)
        for j, tiles_ in enumerate(CHUNKS):
            vals = e2w_g[:, list(tiles_)]
            cnt = np.sum(~np.isnan(vals), axis=1)
            mreal = cnt > 0
            bias[mreal, j] = -beta * np.nanmean(vals[mreal], axis=1)
            npad[:, j] = np.where(mreal, len(tiles_) - cnt, 0)
        bias_f32 = bias.astype(np.float32)
        pad_term = np.exp(bias_f32.astype(np.float64))
        pad_term = pad_term.astype(bf16).astype(np.float64)
        pad_const = np.zeros((len(CELLS), 128), dtype=np.float64)
        for j in range(len(CHUNKS)):
            pad_const[GROUP_OF_CHUNK[j]] += npad[:, j] * pad_term[:, j]
        out.append((grid, bias_f32, pad_const, cls))
    return out


def _prepare(x, ex_feats, ex_labels, w_unconstrained, gamma_unconstrained,
             beta_unconstrained):
    from concourse import mybir

    x = np.asarray(x, dtype=np.float64)
    e = np.asarray(ex_feats, dtype=np.float64)
    labels = np.asarray(ex_labels).astype(np.int64)
    wu = np.asarray(w_unconstrained, dtype=np.float64)

    beta = float(np.log1p(np.exp(np.float64(beta_unconstrained)))) + EPS
    gamma = float(np.log1p(np.exp(np.float64(gamma_unconstrained)))) + EPS
    wexp = np.exp(wu - wu.max())
    w = wexp / wexp.sum() + EPS

    fp8 = _np_dt(mybir, "float8e4")

    x2w = (x * x) @ w                                 # (B,)
    e2w = (e * e) @ w                                 # (N,)
    e8 = np.ascontiguousarray(e.astype(fp8))          # (N, D)

    # xw_t[r, g, s, b] = S * x[b, g*256+s*128+r] * w[...]
    xw = (S_SCALE * (x * w[None, :])).astype(np.float32)
    xw_t = np.ascontiguousarray(
        xw.reshape(B, NG, 2, 128).transpose(3, 1, 2, 0)).astype(fp8)

    packs = _pack(labels, e2w, beta)
    per_core = []
    for cid in range(NCORES):
        grid, bias_f32, pad_const, cls = packs[cid]
        gf8 = e8[grid.clip(0)]                        # (128, NT, D)
        gf8[grid < 0] = fp8(0.0)
        # e_t[r, t, g, s, p] = gf8[p, t, g*256+s*128+r]
        e_t = np.ascontiguousarray(
            gf8.reshape(128, NT, NG, 2, 128).transpose(4, 1, 2, 3, 0))
        per_core.append({"e_t": e_t, "xw_t": xw_t, "bias": bias_f32})
    return per_core, packs, x2w, beta, gamma


def kernel(x, ex_feats, ex_labels, w_unconstrained, gamma_unconstrained,
           beta_unconstrained, _want_results=False, **run_kwargs):
    from concourse.bass_utils import run_bass_kernel_spmd

    per_core, packs, x2w, beta, gamma = _prepare(
        x, ex_feats, ex_labels, w_unconstrained, gamma_unconstrained,
        beta_unconstrained)

    act_scale = float(2.0 * beta / S_SCALE)
    key = round(act_scale, 12)
    if key not in _prog_cache:
        _prog_cache[key] = _build_program(act_scale)
    nc = _prog_cache[key]

    res = run_bass_kernel_spmd(nc, per_core, list(range(NCORES)), **run_kwargs)

    class_sum = np.zeros((B, C), dtype=np.float64)
    ncells = len(CELLS)
    for cid in range(NCORES):
        acc = np.asarray(res.results[cid]["acc"]).astype(np.float64)
        grid, bias_f32, pad_const, cls = packs[cid]
        for gi in range(ncells):
            part = -pad_const[gi][:, None] + sum(
                acc[b] for b in range(NACC) if ACC_CELL[b] == gi)
            for c in range(C):
                mk = cls[:, gi] == c
                if mk.any():
                    class_sum[:, c] += part[mk].sum(axis=0)

    class_sum *= np.exp(-beta * x2w)[:, None]
    logits = (gamma * np.log(class_sum + EPS)).astype(np.float32)
    if _want_results:
        return logits, res
    return logits
